# revision 4
# baseline (speedup 1.0000x reference)
"""Trainium2 Bass kernel for the merged multi-adapter LoRA layer.

Math (all fp32 reference):
    t[n,b,j,d]  = sum_m x[b,j,m] * lora_A[n,d,m]
    out[n,b,j,k] = sum_d t[n,b,j,d] * lora_B[n,k,d]

Shapes: x (4,2048,4096), lora_A (4,16,4096), lora_B (4,4096,16)
        out (4,4,2048,4096)

Sharding: data-parallel over flattened tokens (b*j = 8192 -> 1024/core on
8 cores); the tiny LoRA params are replicated.

Per-core HBM traffic: 8 MiB x (f16 in) + 32 MiB out (f16, widened on host)
+ ~1 MiB params  ->  ~117 us at 358 GB/s.  The schedule is built around
keeping the PE HAM clock-gate warm (2.4 GHz): a cold PE (1.2 GHz) turns the
mm2 stream into the critical path.

Per-core dataflow:
  - x arrives pre-transposed/packed as [chunk, 128, pair, 2*CH] f16; one
    2 MiB DMA per 256-token chunk (16 KiB contiguous per partition).
  - ~14 warm-up matmuls on the (tiny, early-loaded) lora_A tile run while
    chunk 0 streams in, so the HAM un-throttles before real work starts.
  - mm1 (chunk c): t^T[c, tok] accumulated over 32 m-tiles into PSUM;
    chunk 0 runs as one back-to-back burst, chunks 1..3 are interleaved
    into the previous chunk's mm2 stream (after their x DMA has landed) so
    the PE never idles long enough to re-throttle.
  - mm2: out[tok, k] per (128-token strip, adapter); 8 x 512-wide matmuls
    fill four [128,1024] PSUM tiles per output strip; each adapter runs at
    its own 32-row tile_position.
  - PSUM -> SBUF f32->f16 evacuation alternates Vector/Scalar; loads issue
    from GpSimd, stores from Sync, so no engine does double duty.
  - a filler matmul per output strip keeps the PE busy across store/evac
    hiccups (junk result into a scratch PSUM bank).
"""

import numpy as np

import concourse.bacc as bacc
import concourse.bass as bass
import concourse.mybir as mybir
import concourse.tile as tile
from concourse import bass_utils
from concourse.bass import ds, ts

F32 = mybir.dt.float32
F16 = mybir.dt.float16

N_CORES = 8
B, J, M = 4, 2048, 4096
N, D, K = 4, 16, 4096
TOK = B * J                  # 8192 flattened tokens
TPC = TOK // N_CORES         # 1024 tokens per core
CH = 256                     # token chunk (mm1 granularity)
NCH = TPC // CH              # 4
N_MT = M // 128              # 32 m-tiles
NPAIR = N_MT // 2            # 16 packed m-tile pairs
KT = 512                     # mm2 matmul free width (one PSUM bank)
OPW = 1024                   # PSUM evacuation width (two banks)
ADP = 32                     # partition stride per adapter in the packed dim
NSTRIP = CH // 128           # 128-token strips per chunk (2)
WARMUP = 14                  # junk matmuls to un-throttle the PE HAM


def build_program():
    nc = bacc.Bacc("TRN2")

    xs = nc.dram_tensor("xs", [NCH, 128, NPAIR, 2 * CH], F16, kind="ExternalInput").ap()
    a_p = nc.dram_tensor("a_p", [128, N_MT, 128], F16, kind="ExternalInput").ap()
    b_p = nc.dram_tensor("b_p", [128, K], F16, kind="ExternalInput").ap()
    o = nc.dram_tensor("o", [N, TPC, K], F16, kind="ExternalOutput").ap()

    with tile.TileContext(nc) as tc:
        with (
            tc.tile_pool(name="apool", bufs=1) as apool,
            tc.tile_pool(name="bpool", bufs=1) as bpool,
            tc.tile_pool(name="xpool", bufs=NCH) as xpool,
            tc.tile_pool(name="tpool", bufs=2) as tpool,
            tc.tile_pool(name="opool", bufs=8) as opool,
            tc.tile_pool(name="tps", bufs=2, space="PSUM") as tps_pool,
            tc.tile_pool(name="ops", bufs=2, space="PSUM") as ops_pool,
            tc.tile_pool(name="fps", bufs=1, space="PSUM") as fps_pool,
        ):
            a_sb = apool.tile([128, N_MT, 128], F16, tag="a")
            nc.scalar.dma_start(a_sb[:], a_p[:])

            xsb = []
            for c in range(NCH):
                xt = xpool.tile([128, NPAIR, 2 * CH], F16, tag="x", name="x")
                xsb.append(xt)
            nc.scalar.dma_start(xsb[0][:], xs[0])
            b_sb = bpool.tile([128, K], F16, tag="b")
            nc.scalar.dma_start(b_sb[:], b_p[:])
            for c in range(1, NCH):
                nc.scalar.dma_start(xsb[c][:], xs[c])

            f_sc = fps_pool.tile([128, KT], F32, tag="f", name="f")

            def filler(rhs):
                nc.tensor.matmul(f_sc[:], lhsT=a_sb[:, 0, :], rhs=rhs,
                                 start=True, stop=True, skip_group_check=True)

            # HAM warm-up: junk matmuls with no x dependency while chunk 0
            # streams in.  rhs = 4 m-tiles of lora_A (512 wide).
            for w in range(WARMUP):
                filler(a_sb[:, ds(4 * (w % 8), 4), :])

            def mm1(c, mt, t_ps):
                nc.tensor.matmul(
                    t_ps[:],
                    lhsT=a_sb[:, mt, :],
                    rhs=xsb[c][:, mt // 2, ds((mt % 2) * CH, CH)],
                    start=(mt == 0),
                    stop=(mt == N_MT - 1),
                    skip_group_check=True,
                )

            evac = 0
            t_ps_next = tps_pool.tile([128, CH], F32, tag="tps", name="tps")
            for mt in range(N_MT):
                mm1(0, mt, t_ps_next)

            for c in range(NCH):
                t_ps = t_ps_next
                t_sb = tpool.tile([128, CH], F16, tag="t", name="t")
                nc.vector.tensor_copy(t_sb[:], t_ps[:])
                if c + 1 < NCH:
                    t_ps_next = tps_pool.tile([128, CH], F32, tag="tps", name="tps")

                for slot in range(NSTRIP * N):
                    s, n = divmod(slot, N)
                    if c == 0 and slot == 0:
                        # bridge the wait for the b_p DMA right before the
                        # first mm2 so the PE stream stays gap-free
                        filler(xsb[0][:, 0, ds(0, KT)])
                        filler(xsb[0][:, 1, ds(0, KT)])
                    osb = opool.tile([128, K], F16, tag="o", name="osb")
                    for kg in range(K // OPW):
                        o_ps = ops_pool.tile([128, OPW], F32, tag="ops", name="ops")
                        for kk in range(OPW // KT):
                            nc.tensor.matmul(
                                o_ps[:, ts(kk, KT)],
                                lhsT=t_sb[ds(ADP * n, D), ts(s, 128)],
                                rhs=b_sb[ds(ADP * n, D), ds(kg * OPW + kk * KT, KT)],
                                start=True,
                                stop=True,
                                tile_position=(ADP * n, 0),
                                skip_group_check=True,
                            )
                        if evac % 2 == 0:
                            nc.vector.tensor_copy(osb[:, ts(kg, OPW)], o_ps[:])
                        else:
                            nc.scalar.copy(osb[:, ts(kg, OPW)], o_ps[:])
                        evac += 1

                    # interleave next chunk's mm1 into this chunk's mm2
                    # stream; chunk 1's x lands mid-chunk-0, so only use the
                    # back half of chunk 0's slots
                    if c + 1 < NCH:
                        if c == 0:
                            if slot >= 4:
                                for mt in range(8 * (slot - 4), 8 * (slot - 3)):
                                    mm1(1, mt, t_ps_next)
                            else:
                                filler(xsb[0][:, slot, ds(0, KT)])
                        else:
                            for mt in range(4 * slot, 4 * (slot + 1)):
                                mm1(c + 1, mt, t_ps_next)
                            filler(xsb[0][:, slot, ds(0, KT)])
                    else:
                        filler(xsb[0][:, slot, ds(0, KT)])
                        filler(xsb[0][:, slot + 8, ds(0, KT)])

                    nc.sync.dma_start(
                        o[n, ds(c * CH + s * 128, 128), :], osb[:]
                    )

    nc.compile()
    return nc


_NC_CACHE = []


def _get_nc():
    if not _NC_CACHE:
        _NC_CACHE.append(build_program())
    return _NC_CACHE[0]


def prepare_inputs(x, lora_A, lora_B):
    x = np.ascontiguousarray(np.asarray(x, dtype=np.float32)).astype(np.float16)
    lora_A = np.asarray(lora_A, dtype=np.float32)
    lora_B = np.asarray(lora_B, dtype=np.float32)

    xf = x.reshape(TOK, M)

    # a_t[m, 32n+d] = lora_A[n, d, m]; packed to [p, mt, c] so each SBUF
    # partition reads one contiguous row.
    a_t = np.zeros((M, 128), dtype=np.float32)
    for n in range(N):
        a_t[:, ADP * n : ADP * n + D] = lora_A[n].T
    a_pack = np.ascontiguousarray(
        a_t.reshape(N_MT, 128, 128).transpose(1, 0, 2)
    ).astype(np.float16)

    # b_pad[32n+d, k] = lora_B[n, k, d]
    b_pad = np.zeros((128, K), dtype=np.float16)
    for n in range(N):
        b_pad[ADP * n : ADP * n + D, :] = lora_B[n].T

    in_maps = []
    for c in range(N_CORES):
        # xp[chunk, p, pair, sub*CH + t] = x^T[(2*pair+sub)*128 + p,
        #                                      chunk*CH + t]
        xT = xf[c * TPC : (c + 1) * TPC].T              # [M, TPC]
        xr = xT.reshape(NPAIR, 2, 128, NCH, CH)         # [pair, sub, p, ch, t]
        xp = np.ascontiguousarray(xr.transpose(3, 2, 0, 1, 4)).reshape(
            NCH, 128, NPAIR, 2 * CH
        )
        in_maps.append({"xs": xp, "a_p": a_pack, "b_p": b_pad})
    return in_maps


def run(x, lora_A, lora_B, trace=False, **spmd_kwargs):
    nc = _get_nc()
    in_maps = prepare_inputs(x, lora_A, lora_B)
    res = bass_utils.run_bass_kernel_spmd(
        nc, in_maps, list(range(N_CORES)), trace=trace, **spmd_kwargs
    )
    o_full = np.concatenate(
        [res.results[c]["o"].astype(np.float32) for c in range(N_CORES)], axis=1
    )
    return o_full.reshape(N, B, J, K), res


def kernel(x, lora_A, lora_B):
    out, _ = run(x, lora_A, lora_B)
    return out


# revision 6
# speedup vs baseline: 1.0119x; 1.0119x over previous
"""Trainium2 Bass kernel for the merged multi-adapter LoRA layer.

Math (all fp32 reference):
    t[n,b,j,d]  = sum_m x[b,j,m] * lora_A[n,d,m]
    out[n,b,j,k] = sum_d t[n,b,j,d] * lora_B[n,k,d]

Shapes: x (4,2048,4096), lora_A (4,16,4096), lora_B (4,4096,16)
        out (4,4,2048,4096)

Sharding: data-parallel over flattened tokens (b*j = 8192 -> 1024/core on
8 cores); the tiny LoRA params are replicated.

Per-core HBM traffic: 8 MiB x (f16 in) + 32 MiB out (f16, widened on host)
+ ~1 MiB params  ->  ~117 us at 358 GB/s.  The schedule is built around
keeping the PE HAM clock-gate warm (2.4 GHz): a cold PE (1.2 GHz) turns the
mm2 stream into the critical path.

Per-core dataflow:
  - x arrives pre-transposed/packed as [chunk, 128, pair, 2*CH] f16; one
    2 MiB DMA per 256-token chunk (16 KiB contiguous per partition).
  - ~14 warm-up matmuls on the (tiny, early-loaded) lora_A tile run while
    chunk 0 streams in, so the HAM un-throttles before real work starts.
  - mm1 (chunk c): t^T[c, tok] accumulated over 32 m-tiles into PSUM;
    chunk 0 runs as one back-to-back burst, chunks 1..3 are interleaved
    into the previous chunk's mm2 stream (after their x DMA has landed) so
    the PE never idles long enough to re-throttle.
  - mm2: out[tok, k] per (128-token strip, adapter); 8 x 512-wide matmuls
    fill four [128,1024] PSUM tiles per output strip; each adapter runs at
    its own 32-row tile_position.
  - PSUM -> SBUF f32->f16 evacuation alternates Vector/Scalar; loads issue
    from GpSimd, stores from Sync, so no engine does double duty.
  - a filler matmul per output strip keeps the PE busy across store/evac
    hiccups (junk result into a scratch PSUM bank).
"""

import numpy as np

import concourse.bacc as bacc
import concourse.bass as bass
import concourse.mybir as mybir
import concourse.tile as tile
from concourse import bass_utils
from concourse.bass import ds, ts

F32 = mybir.dt.float32
F16 = mybir.dt.float16

N_CORES = 8
B, J, M = 4, 2048, 4096
N, D, K = 4, 16, 4096
TOK = B * J                  # 8192 flattened tokens
TPC = TOK // N_CORES         # 1024 tokens per core
CH = 256                     # token chunk (mm1 granularity)
NCH = TPC // CH              # 4
N_MT = M // 128              # 32 m-tiles
NPAIR = N_MT // 2            # 16 packed m-tile pairs
KT = 512                     # mm2 matmul free width (one PSUM bank)
OPW = 1024                   # PSUM evacuation width (two banks)
ADP = 32                     # partition stride per adapter in the packed dim
NSTRIP = CH // 128           # 128-token strips per chunk (2)
WARMUP = 14                  # junk matmuls to un-throttle the PE HAM


def build_program():
    nc = bacc.Bacc("TRN2")

    xs = nc.dram_tensor("xs", [NCH, 128, NPAIR, 2 * CH], F16, kind="ExternalInput").ap()
    a_p = nc.dram_tensor("a_p", [128, N_MT, 128], F16, kind="ExternalInput").ap()
    b_p = nc.dram_tensor("b_p", [128, K], F16, kind="ExternalInput").ap()
    o = nc.dram_tensor("o", [N, TPC, K], F16, kind="ExternalOutput").ap()

    with tile.TileContext(nc) as tc:
        with (
            tc.tile_pool(name="apool", bufs=1) as apool,
            tc.tile_pool(name="bpool", bufs=1) as bpool,
            tc.tile_pool(name="xpool", bufs=NCH) as xpool,
            tc.tile_pool(name="tpool", bufs=2) as tpool,
            tc.tile_pool(name="opool", bufs=10) as opool,
            tc.tile_pool(name="tps", bufs=2, space="PSUM") as tps_pool,
            tc.tile_pool(name="ops", bufs=5, space="PSUM") as ops_pool,
            tc.tile_pool(name="fps", bufs=1, space="PSUM") as fps_pool,
        ):
            a_sb = apool.tile([128, N_MT, 128], F16, tag="a")
            nc.scalar.dma_start(a_sb[:], a_p[:])

            xsb = []
            for c in range(NCH):
                xt = xpool.tile([128, NPAIR, 2 * CH], F16, tag="x", name="x")
                xsb.append(xt)
            nc.scalar.dma_start(xsb[0][:], xs[0])
            b_sb = bpool.tile([128, K], F16, tag="b")
            nc.scalar.dma_start(b_sb[:], b_p[:])
            for c in range(1, NCH):
                nc.scalar.dma_start(xsb[c][:], xs[c])

            f_sc = fps_pool.tile([128, KT], F32, tag="f", name="f")

            def filler(rhs):
                nc.tensor.matmul(f_sc[:], lhsT=a_sb[:, 0, :], rhs=rhs,
                                 start=True, stop=True, skip_group_check=True)

            # HAM warm-up: junk matmuls with no x dependency while chunk 0
            # streams in.  rhs = 4 m-tiles of lora_A (512 wide).
            for w in range(WARMUP):
                filler(a_sb[:, ds(4 * (w % 8), 4), :])

            def mm1(c, mt, t_ps):
                nc.tensor.matmul(
                    t_ps[:],
                    lhsT=a_sb[:, mt, :],
                    rhs=xsb[c][:, mt // 2, ds((mt % 2) * CH, CH)],
                    start=(mt == 0),
                    stop=(mt == N_MT - 1),
                    skip_group_check=True,
                )

            evac = 0
            pad_i = 0

            def pad(k):
                # junk matmuls that absorb what would otherwise be PE
                # dependency waits, so the HAM never sees an idle window
                nonlocal pad_i
                for _ in range(k):
                    filler(xsb[0][:, pad_i % NPAIR, ds(0, KT)])
                    pad_i += 1

            t_ps_next = tps_pool.tile([128, CH], F32, tag="tps", name="tps")
            for mt in range(N_MT):
                mm1(0, mt, t_ps_next)
            t_sb_next = tpool.tile([128, CH], F16, tag="t", name="t")
            nc.vector.tensor_copy(t_sb_next[:], t_ps_next[:])

            for c in range(NCH):
                t_sb = t_sb_next
                if c + 1 < NCH:
                    t_ps_next = tps_pool.tile([128, CH], F32, tag="tps", name="tps")

                for slot in range(NSTRIP * N):
                    s, n = divmod(slot, N)
                    if c == 0 and slot == 0:
                        # bridge the wait for the b_p DMA right before the
                        # first mm2 so the PE stream stays gap-free
                        pad(2)
                    osb = opool.tile([128, K], F16, tag="o", name="osb")

                    # next chunk's mm1 matmuls woven into this chunk's mm2
                    # stream; chunk 1's x lands mid-chunk-0, so only the back
                    # half of chunk 0's slots carry mm1 work
                    mm1_mts = []
                    if c + 1 < NCH:
                        if c == 0:
                            if slot >= 4:
                                mm1_mts = list(range(8 * (slot - 4), 8 * (slot - 3)))
                        else:
                            mm1_mts = list(range(4 * slot, 4 * (slot + 1)))
                    n_pad = {0: 5, 4: 3, 8: 2}[len(mm1_mts)]

                    for kg in range(K // KT):
                        o_ps = ops_pool.tile([128, KT], F32, tag="ops", name="ops")
                        nc.tensor.matmul(
                            o_ps[:],
                            lhsT=t_sb[ds(ADP * n, D), ts(s, 128)],
                            rhs=b_sb[ds(ADP * n, D), ts(kg, KT)],
                            start=True,
                            stop=True,
                            tile_position=(ADP * n, 0),
                            skip_group_check=True,
                        )
                        if evac % 2 == 0:
                            nc.vector.tensor_copy(osb[:, ts(kg, KT)], o_ps[:])
                        else:
                            nc.scalar.copy(osb[:, ts(kg, KT)], o_ps[:])
                        evac += 1
                        # weave mm1 + pad into the mm2 stream
                        if kg == 3 or kg == 7:
                            half = mm1_mts[: len(mm1_mts) // 2] if kg == 3 \
                                else mm1_mts[len(mm1_mts) // 2 :]
                            for mt in half:
                                mm1(c + 1, mt, t_ps_next)
                                if mt == N_MT - 1:
                                    t_sb_next = tpool.tile(
                                        [128, CH], F16, tag="t", name="t"
                                    )
                                    nc.vector.tensor_copy(
                                        t_sb_next[:], t_ps_next[:]
                                    )
                            pad(n_pad - n_pad // 2 if kg == 3 else n_pad // 2)

                    nc.sync.dma_start(
                        o[n, ds(c * CH + s * 128, 128), :], osb[:]
                    )

    nc.compile()
    return nc


_NC_CACHE = []


def _get_nc():
    if not _NC_CACHE:
        _NC_CACHE.append(build_program())
    return _NC_CACHE[0]


def prepare_inputs(x, lora_A, lora_B):
    x = np.ascontiguousarray(np.asarray(x, dtype=np.float32)).astype(np.float16)
    lora_A = np.asarray(lora_A, dtype=np.float32)
    lora_B = np.asarray(lora_B, dtype=np.float32)

    xf = x.reshape(TOK, M)

    # a_t[m, 32n+d] = lora_A[n, d, m]; packed to [p, mt, c] so each SBUF
    # partition reads one contiguous row.
    a_t = np.zeros((M, 128), dtype=np.float32)
    for n in range(N):
        a_t[:, ADP * n : ADP * n + D] = lora_A[n].T
    a_pack = np.ascontiguousarray(
        a_t.reshape(N_MT, 128, 128).transpose(1, 0, 2)
    ).astype(np.float16)

    # b_pad[32n+d, k] = lora_B[n, k, d]
    b_pad = np.zeros((128, K), dtype=np.float16)
    for n in range(N):
        b_pad[ADP * n : ADP * n + D, :] = lora_B[n].T

    in_maps = []
    for c in range(N_CORES):
        # xp[chunk, p, pair, sub*CH + t] = x^T[(2*pair+sub)*128 + p,
        #                                      chunk*CH + t]
        xT = xf[c * TPC : (c + 1) * TPC].T              # [M, TPC]
        xr = xT.reshape(NPAIR, 2, 128, NCH, CH)         # [pair, sub, p, ch, t]
        xp = np.ascontiguousarray(xr.transpose(3, 2, 0, 1, 4)).reshape(
            NCH, 128, NPAIR, 2 * CH
        )
        in_maps.append({"xs": xp, "a_p": a_pack, "b_p": b_pad})
    return in_maps


def run(x, lora_A, lora_B, trace=False, **spmd_kwargs):
    nc = _get_nc()
    in_maps = prepare_inputs(x, lora_A, lora_B)
    res = bass_utils.run_bass_kernel_spmd(
        nc, in_maps, list(range(N_CORES)), trace=trace, **spmd_kwargs
    )
    o_full = np.concatenate(
        [res.results[c]["o"].astype(np.float32) for c in range(N_CORES)], axis=1
    )
    return o_full.reshape(N, B, J, K), res


def kernel(x, lora_A, lora_B):
    out, _ = run(x, lora_A, lora_B)
    return out


# revision 8
# speedup vs baseline: 1.3127x; 1.2973x over previous
"""Trainium2 Bass kernel for the merged multi-adapter LoRA layer.

Math (all fp32 reference):
    t[n,b,j,d]  = sum_m x[b,j,m] * lora_A[n,d,m]
    out[n,b,j,k] = sum_d t[n,b,j,d] * lora_B[n,k,d]

Shapes: x (4,2048,4096), lora_A (4,16,4096), lora_B (4,4096,16)
        out (4,4,2048,4096)

Sharding: data-parallel over flattened tokens (b*j = 8192 -> 1024/core on
8 cores); the tiny LoRA params are replicated.

Per-core HBM traffic: 8 MiB x (f16 in) + 32 MiB out (f16, widened on host)
+ ~1 MiB params  ->  ~117 us at 358 GB/s.  The schedule is built around
keeping the PE HAM clock-gate warm (2.4 GHz): a cold PE (1.2 GHz) turns the
mm2 stream into the critical path.

Per-core dataflow:
  - x arrives pre-transposed/packed as [chunk, 128, pair, 2*CH] f16; one
    2 MiB DMA per 256-token chunk (16 KiB contiguous per partition).
  - ~14 warm-up matmuls on the (tiny, early-loaded) lora_A tile run while
    chunk 0 streams in, so the HAM un-throttles before real work starts.
  - mm1 (chunk c): t^T[c, tok] accumulated over 32 m-tiles into PSUM;
    chunk 0 runs as one back-to-back burst, chunks 1..3 are interleaved
    into the previous chunk's mm2 stream (after their x DMA has landed) so
    the PE never idles long enough to re-throttle.
  - mm2: out[tok, k] per (128-token strip, adapter); 8 x 512-wide matmuls
    fill four [128,1024] PSUM tiles per output strip; each adapter runs at
    its own 32-row tile_position.
  - PSUM -> SBUF f32->f16 evacuation alternates Vector/Scalar; loads issue
    from GpSimd, stores from Sync, so no engine does double duty.
  - a filler matmul per output strip keeps the PE busy across store/evac
    hiccups (junk result into a scratch PSUM bank).
"""

import numpy as np

import concourse.bacc as bacc
import concourse.bass as bass
import concourse.mybir as mybir
import concourse.tile as tile
from concourse import bass_utils
from concourse.bass import ds, ts

F32 = mybir.dt.float32
F16 = mybir.dt.float16

N_CORES = 8
B, J, M = 4, 2048, 4096
N, D, K = 4, 16, 4096
TOK = B * J                  # 8192 flattened tokens
TPC = TOK // N_CORES         # 1024 tokens per core
CH = 256                     # token chunk (mm1 granularity)
NCH = TPC // CH              # 4
N_MT = M // 128              # 32 m-tiles
NPAIR = N_MT // 2            # 16 packed m-tile pairs
KT = 512                     # mm2 matmul free width (one PSUM bank)
OPW = 1024                   # PSUM evacuation width (two banks)
ADP = 32                     # partition stride per adapter in the packed dim
NSTRIP = CH // 128           # 128-token strips per chunk (2)
WARMUP = 14                  # junk matmuls to un-throttle the PE HAM


def build_program():
    nc = bacc.Bacc("TRN2")

    xs = nc.dram_tensor("xs", [NCH, 128, NPAIR, 2 * CH], F16, kind="ExternalInput").ap()
    a_p = nc.dram_tensor("a_p", [128, N_MT, 128], F16, kind="ExternalInput").ap()
    b_p = nc.dram_tensor("b_p", [128, K], F16, kind="ExternalInput").ap()
    o = nc.dram_tensor("o", [N, TPC, K], F16, kind="ExternalOutput").ap()

    with tile.TileContext(nc) as tc:
        with (
            tc.tile_pool(name="apool", bufs=1) as apool,
            tc.tile_pool(name="bpool", bufs=1) as bpool,
            tc.tile_pool(name="xpool", bufs=NCH) as xpool,
            tc.tile_pool(name="tpool", bufs=2) as tpool,
            tc.tile_pool(name="opool", bufs=13) as opool,
            tc.tile_pool(name="tps", bufs=2, space="PSUM") as tps_pool,
            tc.tile_pool(name="ops", bufs=5, space="PSUM") as ops_pool,
            tc.tile_pool(name="fps", bufs=1, space="PSUM") as fps_pool,
        ):
            a_sb = apool.tile([128, N_MT, 128], F16, tag="a")
            nc.scalar.dma_start(a_sb[:], a_p[:])

            xsb = []
            for c in range(NCH):
                xt = xpool.tile([128, NPAIR, 2 * CH], F16, tag="x", name="x")
                xsb.append(xt)
            nc.scalar.dma_start(xsb[0][:], xs[0])
            b_sb = bpool.tile([128, K], F16, tag="b")
            nc.scalar.dma_start(b_sb[:], b_p[:])
            for c in range(1, NCH):
                nc.scalar.dma_start(xsb[c][:], xs[c])

            f_sc = fps_pool.tile([128, KT], F32, tag="f", name="f")

            def filler(rhs):
                nc.tensor.matmul(f_sc[:], lhsT=a_sb[:, 0, :], rhs=rhs,
                                 start=True, stop=True, skip_group_check=True)

            # HAM warm-up: junk matmuls with no x dependency while chunk 0
            # streams in.  rhs = 4 m-tiles of lora_A (512 wide).
            for w in range(WARMUP):
                filler(a_sb[:, ds(4 * (w % 8), 4), :])

            def mm1(c, mt, t_ps):
                nc.tensor.matmul(
                    t_ps[:],
                    lhsT=a_sb[:, mt, :],
                    rhs=xsb[c][:, mt // 2, ds((mt % 2) * CH, CH)],
                    start=(mt == 0),
                    stop=(mt == N_MT - 1),
                    skip_group_check=True,
                )

            evac = 0
            pad_i = 0

            def pad(k):
                # junk matmuls that absorb what would otherwise be PE
                # dependency waits, so the HAM never sees an idle window
                nonlocal pad_i
                for _ in range(k):
                    filler(xsb[0][:, pad_i % NPAIR, ds(0, KT)])
                    pad_i += 1

            t_ps_next = tps_pool.tile([128, CH], F32, tag="tps", name="tps")
            for mt in range(N_MT):
                mm1(0, mt, t_ps_next)
            t_sb_next = tpool.tile([128, CH], F16, tag="t", name="t")
            nc.vector.tensor_copy(t_sb_next[:], t_ps_next[:])

            for c in range(NCH):
                t_sb = t_sb_next
                if c + 1 < NCH:
                    t_ps_next = tps_pool.tile([128, CH], F32, tag="tps", name="tps")

                for slot in range(NSTRIP * N):
                    s, n = divmod(slot, N)
                    if c == 0 and slot == 0:
                        # bridge the wait for the b_p DMA right before the
                        # first mm2 so the PE stream stays gap-free
                        pad(2)
                    osb = opool.tile([128, K], F16, tag="o", name="osb")

                    # next chunk's mm1 matmuls woven into this chunk's mm2
                    # stream; chunk 1's x lands mid-chunk-0, so only the back
                    # half of chunk 0's slots carry mm1 work
                    mm1_mts = []
                    if c + 1 < NCH:
                        if c == 0:
                            if slot >= 4:
                                mm1_mts = list(range(8 * (slot - 4), 8 * (slot - 3)))
                        else:
                            mm1_mts = list(range(4 * slot, 4 * (slot + 1)))
                    n_pad = 0

                    for kg in range(K // KT):
                        o_ps = ops_pool.tile([128, KT], F32, tag="ops", name="ops")
                        nc.tensor.matmul(
                            o_ps[:],
                            lhsT=t_sb[ds(ADP * n, D), ts(s, 128)],
                            rhs=b_sb[ds(ADP * n, D), ts(kg, KT)],
                            start=True,
                            stop=True,
                            tile_position=(ADP * n, 0),
                            skip_group_check=True,
                        )
                        if evac % 2 == 0:
                            nc.vector.tensor_copy(osb[:, ts(kg, KT)], o_ps[:])
                        else:
                            nc.scalar.copy(osb[:, ts(kg, KT)], o_ps[:])
                        evac += 1
                        # weave mm1 + pad into the mm2 stream
                        if kg == 3 or kg == 7:
                            half = mm1_mts[: len(mm1_mts) // 2] if kg == 3 \
                                else mm1_mts[len(mm1_mts) // 2 :]
                            for mt in half:
                                mm1(c + 1, mt, t_ps_next)
                                if mt == N_MT - 1:
                                    t_sb_next = tpool.tile(
                                        [128, CH], F16, tag="t", name="t"
                                    )
                                    nc.vector.tensor_copy(
                                        t_sb_next[:], t_ps_next[:]
                                    )
                            pad(n_pad - n_pad // 2 if kg == 3 else n_pad // 2)

                    nc.sync.dma_start(
                        o[n, ds(c * CH + s * 128, 128), :], osb[:]
                    )

    nc.compile()
    return nc


_NC_CACHE = []


def _get_nc():
    if not _NC_CACHE:
        _NC_CACHE.append(build_program())
    return _NC_CACHE[0]


def prepare_inputs(x, lora_A, lora_B):
    x = np.ascontiguousarray(np.asarray(x, dtype=np.float32)).astype(np.float16)
    lora_A = np.asarray(lora_A, dtype=np.float32)
    lora_B = np.asarray(lora_B, dtype=np.float32)

    xf = x.reshape(TOK, M)

    # a_t[m, 32n+d] = lora_A[n, d, m]; packed to [p, mt, c] so each SBUF
    # partition reads one contiguous row.
    a_t = np.zeros((M, 128), dtype=np.float32)
    for n in range(N):
        a_t[:, ADP * n : ADP * n + D] = lora_A[n].T
    a_pack = np.ascontiguousarray(
        a_t.reshape(N_MT, 128, 128).transpose(1, 0, 2)
    ).astype(np.float16)

    # b_pad[32n+d, k] = lora_B[n, k, d]
    b_pad = np.zeros((128, K), dtype=np.float16)
    for n in range(N):
        b_pad[ADP * n : ADP * n + D, :] = lora_B[n].T

    in_maps = []
    for c in range(N_CORES):
        # xp[chunk, p, pair, sub*CH + t] = x^T[(2*pair+sub)*128 + p,
        #                                      chunk*CH + t]
        xT = xf[c * TPC : (c + 1) * TPC].T              # [M, TPC]
        xr = xT.reshape(NPAIR, 2, 128, NCH, CH)         # [pair, sub, p, ch, t]
        xp = np.ascontiguousarray(xr.transpose(3, 2, 0, 1, 4)).reshape(
            NCH, 128, NPAIR, 2 * CH
        )
        in_maps.append({"xs": xp, "a_p": a_pack, "b_p": b_pad})
    return in_maps


def run(x, lora_A, lora_B, trace=False, **spmd_kwargs):
    nc = _get_nc()
    in_maps = prepare_inputs(x, lora_A, lora_B)
    res = bass_utils.run_bass_kernel_spmd(
        nc, in_maps, list(range(N_CORES)), trace=trace, **spmd_kwargs
    )
    o_full = np.concatenate(
        [res.results[c]["o"].astype(np.float32) for c in range(N_CORES)], axis=1
    )
    return o_full.reshape(N, B, J, K), res


def kernel(x, lora_A, lora_B):
    out, _ = run(x, lora_A, lora_B)
    return out
